# revision 8
# baseline (speedup 1.0000x reference)
# Bass/Trainium2 kernel for nn_AA2_Module_75359496175785 (sparse_attention).
#
# Math (per batch item b; x: (C,N) with C=128, N=H*W=16384):
#   q  = Wq x + bq;  k_g = Wk_g pool(x) + bk_g   (pooling commutes with 1x1 conv)
#   e_g = q^T k_g;   a_g = softmax(alpha_g e_g, axis=keys)
#   out = gamma0 k_0 a_0^T + x + gamma1 k_1 a_1^T
#
# Key structure vs the naive version:
#  * alpha/256 folded into Wk host-side; gamma/alpha folded into the kT scale.
#  * M = Wq^T k_cat is further expanded host-side: M = (Wks^T Wq)^T pool(x) + mb,
#    so M comes straight from the pooled x with no serial q/k chain.
#  * Energy consumes x directly as float32r (fp22) moving operand - at free
#    dim >= 256 this runs at bf16 speed, so x never needs a bf16 cast pass.
#  * Phase 0 is DMA-bound: the only compute is the DVE pool-reduce per chunk.
#    PE is warmed with dummy transposes so phase 1 runs at 2.4 GHz from the
#    first energy matmul (HAM clock gate).
#  * Phase 1 is a 3-stage software pipeline over 1024-column groups:
#      PE : energy(g) | sums(g-1) | out(g-2)+x-add(idb, half)
#      ACT: exp(g)    | psum->sbuf copy(g-2) (half)
#      DVE: recip(g-1), attn-mul tail(g-1), stt copy+x-add(g-2) (half)
#      GPS: attn-mul head(g-1)
import numpy as np

B, C, H, W = 8, 128, 128, 128
N = H * W
PP = 8
NKEYS = 64
CHUNK = 2048      # phase-0 dma chunk = 16 rows of H (one pool-block row)
NCHUNK = N // CHUNK
GRP = 512
PAIR = 2 * GRP    # phase-1 iteration width
NPAIR = N // PAIR
TT_G = 768        # columns (of each 1024) whose attn-multiply runs on gpsimd
NWARM = 20        # dummy transposes to warm the PE clock gate

_CACHE = {}


def _build_nc():
    import concourse.bass as bass  # noqa: F401
    from concourse import bacc, mybir
    import concourse.tile as tile

    f32 = mybir.dt.float32
    f32r = mybir.dt.float32r
    bf16 = mybir.dt.bfloat16
    AF = mybir.ActivationFunctionType

    nc = bacc.Bacc(None, target_bir_lowering=False)

    x_d = nc.dram_tensor("x", [C, N], f32r, kind="ExternalInput")
    # bf16 weights: [WmT0 | WmT1 | WksT0 | WksT1 | idb | ones_bd]
    wb_d = nc.dram_tensor("wb", [C, 6 * C], bf16, kind="ExternalInput")
    # f32 weights: [mb0 mb1 bks0 bks1 bq gvec]
    wf_d = nc.dram_tensor("wf", [C, 6], f32, kind="ExternalInput")
    wi_d = nc.dram_tensor("wi", [C, C], f32r, kind="ExternalInput")
    out_d = nc.dram_tensor("out", [C, N], f32, kind="ExternalOutput")

    with tile.TileContext(nc) as tc:
        with (
            tc.tile_pool(name="const", bufs=1) as const,
            tc.tile_pool(name="big", bufs=1) as big,
            tc.tile_pool(name="expp", bufs=4) as expp,
            tc.tile_pool(name="rp", bufs=3) as rp,
            tc.tile_pool(name="attnp", bufs=3) as attnp,
            tc.tile_pool(name="outp", bufs=5) as outp,
        ):
            wb = const.tile([C, 6 * C], bf16)
            wf = const.tile([C, 6], f32)
            wi = const.tile([C, C], f32r)
            nc.sync.dma_start(wb[:], wb_d[:])
            nc.sync.dma_start(wf[:], wf_d[:])
            nc.sync.dma_start(wi[:], wi_d[:])
            wmT0 = wb[:, 0:C]
            wmT1 = wb[:, C:2 * C]
            wksT0 = wb[:, 2 * C:3 * C]
            wksT1 = wb[:, 3 * C:4 * C]
            idb = wb[:, 4 * C:5 * C]
            ones_bd = wb[:, 5 * C:6 * C]
            mb0 = wf[:, 0:1]
            mb1 = wf[:, 1:2]
            bks0 = wf[:, 2:3]
            bks1 = wf[:, 3:4]
            bq = wf[:, 4:5]
            gvec = wf[:, 5:6]

            x_sb = big.tile([C, N], f32r)
            xp = big.tile([C, NKEYS], f32)
            xp_bf = big.tile([C, NKEYS], bf16)
            m_sb = big.tile([C, C], f32r)
            k_bf = big.tile([C, C], bf16)
            kT = big.tile([C, C], bf16)
            ebias = big.tile([C, 1], f32)
            bq_bf = big.tile([C, 1], bf16)
            tiny = big.tile([C, 1], f32)

            def xr(sl):
                return x_sb[:, sl]

            def xf(sl):
                return x_sb[:, sl].bitcast(f32)

            # ---- phase 0: stream x, pool on DVE, keep PE warm ----
            ph0 = tc.tile_pool(name="ps0", bufs=1, space="PSUM")
            ps0 = ph0.__enter__()
            scr_bf = ps0.tile([C, C], bf16, tag="scrb")
            scr_f = ps0.tile([C, C], f32r, tag="scrf")

            # preload the exp table set + prime engines (gated on wf arrival)
            nc.scalar.activation(tiny[:], wf[:, 0:1], AF.Exp)
            nc.vector.tensor_copy(bq_bf[:], bq)
            # warm the PE clock gate: ~20 back-to-back transposes (~4us cold)
            for i in range(NWARM):
                nc.tensor.transpose(scr_bf[:], wb[:, 0:C], idb)

            for c in range(NCHUNK):
                csl = bass.ts(c, CHUNK)
                nc.sync.dma_start(x_sb[:, csl], x_d[:, csl])
                xc = x_sb[:, csl].bitcast(f32).rearrange(
                    "p (h pj w) -> p pj h w", h=16, pj=PP, w=16
                )
                nc.vector.tensor_reduce(
                    xp[:, c * PP:(c + 1) * PP], xc,
                    axis=mybir.AxisListType.XY, op=mybir.AluOpType.add,
                )
                # keep the PE busy-window alive until the next chunk lands
                nc.tensor.transpose(
                    scr_f[:], x_sb[:, bass.ds(c * CHUNK, C)], wi
                )

            # ---- mid: M (energy weights), keys, kT, ebias ----
            m_ps = ps0.tile([C, C], f32, tag="mps")
            kk_ps = ps0.tile([C, C], f32, tag="kkps")
            kT_ps = ps0.tile([C, C], bf16, tag="ktps")
            eb_ps = ps0.tile([C, 1], f32, tag="ebps")

            nc.vector.tensor_copy(xp_bf[:], xp[:])
            nc.tensor.matmul(m_ps[:, 0:NKEYS], wmT0, xp_bf[:], start=True, stop=True)
            nc.tensor.matmul(m_ps[:, NKEYS:], wmT1, xp_bf[:], start=True, stop=True)
            nc.scalar.activation(
                m_sb[:, 0:NKEYS], m_ps[:, 0:NKEYS], AF.Identity, bias=mb0, scale=1.0
            )
            nc.scalar.activation(
                m_sb[:, NKEYS:], m_ps[:, NKEYS:], AF.Identity, bias=mb1, scale=1.0
            )
            nc.tensor.matmul(kk_ps[:, 0:NKEYS], wksT0, xp_bf[:], start=True, stop=True)
            nc.tensor.matmul(kk_ps[:, NKEYS:], wksT1, xp_bf[:], start=True, stop=True)
            nc.scalar.activation(
                k_bf[:, 0:NKEYS], kk_ps[:, 0:NKEYS], AF.Identity, bias=bks0, scale=1.0
            )
            nc.scalar.activation(
                k_bf[:, NKEYS:], kk_ps[:, NKEYS:], AF.Identity, bias=bks1, scale=1.0
            )
            nc.tensor.matmul(eb_ps[:], k_bf[:], bq_bf[:], start=True, stop=True)
            nc.tensor.transpose(kT_ps[:], k_bf[:], idb)
            nc.vector.tensor_copy(ebias[:], eb_ps[:])
            nc.scalar.activation(kT[:], kT_ps[:], AF.Copy, scale=gvec)
            ph0.__exit__(None, None, None)

            # ---- phase 1: 3-stage pipelined groups of 1024 columns ----
            ph_e = tc.tile_pool(name="ps_e", bufs=2, space="PSUM")
            ps_e = ph_e.__enter__()
            ph_s = tc.tile_pool(name="ps_s", bufs=1, space="PSUM")
            ps_s = ph_s.__enter__()
            ph_u = tc.tile_pool(name="ps_u", bufs=1, space="PSUM")
            ps_u = ph_u.__enter__()

            exps = [None] * NPAIR
            attns = [None] * NPAIR
            for g in range(NPAIR + 2):
                ga, gb, gc = g, g - 1, g - 2
                if ga < NPAIR:
                    a0 = bass.ds(ga * PAIR, GRP)
                    a1 = bass.ds(ga * PAIR + GRP, GRP)
                    e_ps = ps_e.tile([C, PAIR], f32, tag="eps")
                    nc.tensor.matmul(e_ps[:, 0:GRP], m_sb[:], xr(a0), start=True, stop=True)
                    nc.tensor.matmul(e_ps[:, GRP:], m_sb[:], xr(a1), start=True, stop=True)
                    exps[ga] = expp.tile([C, PAIR], bf16, tag="exp", name="exp_sb")
                    nc.scalar.activation(
                        exps[ga][:], e_ps[:], AF.Exp, bias=ebias[:, 0:1], scale=1.0
                    )
                if 0 <= gb < NPAIR:
                    s_ps = ps_s.tile([C, PAIR], f32, tag="sps")
                    nc.tensor.matmul(
                        s_ps[:, 0:GRP], ones_bd, exps[gb][:, 0:GRP], start=True, stop=True
                    )
                    nc.tensor.matmul(
                        s_ps[:, GRP:], ones_bd, exps[gb][:, GRP:], start=True, stop=True
                    )
                    r_sb = rp.tile([C, PAIR], f32)
                    nc.vector.reciprocal_approx_fast(out=r_sb[:], in_=s_ps[:])
                    attns[gb] = attnp.tile([C, PAIR], bf16, tag="attn", name="attn_sb")
                    nc.gpsimd.tensor_mul(
                        attns[gb][:, 0:TT_G], exps[gb][:, 0:TT_G], r_sb[:, 0:TT_G]
                    )
                    nc.vector.tensor_mul(
                        attns[gb][:, TT_G:], exps[gb][:, TT_G:], r_sb[:, TT_G:]
                    )
                if 0 <= gc < NPAIR:
                    c0 = bass.ds(gc * PAIR, GRP)
                    c1 = bass.ds(gc * PAIR + GRP, GRP)
                    u_ps = ps_u.tile([C, PAIR], f32, tag="ups")
                    nc.tensor.matmul(
                        u_ps[:, 0:GRP], kT[:], attns[gc][:, 0:GRP],
                        start=True, stop=False,
                    )
                    nc.tensor.matmul(
                        u_ps[:, 0:GRP], wi[:], xr(c0), start=False, stop=True
                    )
                    nc.tensor.matmul(
                        u_ps[:, GRP:], kT[:], attns[gc][:, GRP:],
                        start=True, stop=True,
                    )
                    o_sb = outp.tile([C, PAIR], f32)
                    nc.scalar.activation(o_sb[:, 0:GRP], u_ps[:, 0:GRP], AF.Copy)
                    nc.vector.scalar_tensor_tensor(
                        out=o_sb[:, GRP:],
                        in0=u_ps[:, GRP:],
                        scalar=1.0,
                        in1=xf(c1),
                        op0=mybir.AluOpType.mult,
                        op1=mybir.AluOpType.add,
                    )
                    nc.sync.dma_start(out_d[:, bass.ds(gc * PAIR, PAIR)], o_sb[:])
            ph_u.__exit__(None, None, None)
            ph_s.__exit__(None, None, None)
            ph_e.__exit__(None, None, None)

    nc.compile()
    return nc


def _get_nc():
    if "nc" not in _CACHE:
        _CACHE["nc"] = _build_nc()
    return _CACHE["nc"]


def _make_in_maps(x, Wq, bq, Wk, bk, Wk1, bk1, gamma, gamma1, aphal, aphal1):
    a0 = float(np.asarray(aphal).reshape(-1)[0])
    a1 = float(np.asarray(aphal1).reshape(-1)[0])
    g0 = float(np.asarray(gamma).reshape(-1)[0])
    g1 = float(np.asarray(gamma1).reshape(-1)[0])

    f = np.float32
    Wq = np.asarray(Wq, f)
    Wks0 = np.asarray(Wk, f) * (a0 / 256.0)
    Wks1 = np.asarray(Wk1, f) * (a1 / 256.0)
    bks0 = np.asarray(bk, f).reshape(C) * a0
    bks1 = np.asarray(bk1, f).reshape(C) * a1
    wmT0 = Wks0.T @ Wq           # stationary for M = (Wq^T Wks) pool(x)
    wmT1 = Wks1.T @ Wq
    mb0 = Wq.T @ bks0
    mb1 = Wq.T @ bks1
    eye = np.eye(C, dtype=f)
    ones_bd = np.kron(np.eye(2, dtype=f), np.ones((NKEYS, NKEYS), f))
    wb = np.concatenate(
        [wmT0, wmT1, Wks0.T, Wks1.T, eye, ones_bd], axis=1
    ).astype("bfloat16")
    gvec = np.concatenate(
        [np.full((NKEYS, 1), g0 / a0, f), np.full((NKEYS, 1), g1 / a1, f)]
    )
    wf = np.concatenate(
        [
            mb0.reshape(C, 1), mb1.reshape(C, 1),
            bks0.reshape(C, 1), bks1.reshape(C, 1),
            np.asarray(bq, f).reshape(C, 1),
            gvec,
        ],
        axis=1,
    ).astype(f)
    wb = np.ascontiguousarray(wb)
    wf = np.ascontiguousarray(wf)
    in_maps = []
    for b in range(B):
        in_maps.append({
            "x": np.ascontiguousarray(np.asarray(x)[b].reshape(C, N), dtype=f),
            "wb": wb,
            "wf": wf,
            "wi": eye,
        })
    return in_maps


def kernel(x, Wq, bq, Wk, bk, Wk1, bk1, gamma, gamma1, aphal, aphal1, **_):
    import ml_dtypes  # noqa: F401
    from concourse.bass_utils import run_bass_kernel_spmd

    nc = _get_nc()
    in_maps = _make_in_maps(
        np.asarray(x), np.asarray(Wq), np.asarray(bq), np.asarray(Wk),
        np.asarray(bk), np.asarray(Wk1), np.asarray(bk1), np.asarray(gamma),
        np.asarray(gamma1), np.asarray(aphal), np.asarray(aphal1),
    )
    res = None
    last_exc = None
    for _attempt in range(3):
        try:
            res = run_bass_kernel_spmd(nc, in_maps, core_ids=list(range(B)))
            break
        except Exception as e:  # transient NRT_EXEC_UNIT_UNRECOVERABLE faults
            last_exc = e
            import time as _time
            _time.sleep(2.0)
    if res is None:
        raise last_exc
    out = np.stack([res.results[b]["out"].reshape(C, H, W) for b in range(B)])
    return out.astype(np.float32)
